# revision 26
# baseline (speedup 1.0000x reference)
"""GPT-2 (L=8, D=1024, H=16, V=50257, B=4, T=1024) forward on 8 TRN2 NeuronCores.

Sharding: core c handles batch b=c//2, sequence half h=c%2 (512 tokens).
Weights replicated (bf16). Per layer, the pair exchanges K/V via a gated
AllGather; slot 0 of the gather is the half-0 member's K/V, which is the
remote half every core's phase B uses (self-masked via the exp bias on
half-0 cores: exp(s - 30000) == 0).

Perf rework vs the original baseline (5.17ms -> ~3.6ms):
- attention phase A causally trimmed: S/exp/AV only computed for q >= kt*128;
  mask multiply only on the [128,128] diagonal block of each key chunk.
- exp batched: 2 activation instructions per head-phase reading multi-bank
  PSUM spans instead of 4; attention heads processed in row-complementary
  pairs so their K=64 S matmuls run concurrently in the PE array.
- ALL dense GEMM chains (K/Q/O proj, MLP, LM head) interleave two output
  chunks across two PSUM banks: back-to-back matmuls accumulating into the
  same bank serialize with their LDWEIGHTS (~320ns/MM for N=512); alternating
  banks hides both and reaches ~233ns/MM.
- keepwarm matmul bursts bridge the K/V-exchange wait (~25us) and the LN /
  denominator serial chains: any PE idle gap > ~3.4us makes the HAM clock
  gate halve the PE clock, which previously kept entire attention phases at
  1.2GHz (measured 1.45ms at K=4/8 -> 0.75ms with the bursts).
- LN chain: fused DVE ops + reciprocal_approx_fast instead of Sqrt+DVE
  reciprocal (iterative divide, 3-4us on the critical path); LN stats
  accumulate into spare PSUM banks via ones-matmuls during the producing
  GEMM's drain loop.
- softmax denominators: reciprocal_approx_fast, bf16 broadcast matmuls,
  explicit tile_position=(96,0) for the 4th head of each den group.
- LM head: vocab tiles in pairs sharing the stationary xf chunk, bf16 logits
  (cast to f32 on host) halving the output write traffic.
- vst ones-columns memset once; PSUM->SBUF drains spread across engines.
Run-to-run note: the chip toggles between 2.4GHz and a ~2.0GHz P0 power
state; identical kernels measure 3.64-3.87ms depending on that state.
"""

import os
import sys
import types

import numpy as np
import ml_dtypes

import concourse.bass as bass
import concourse.mybir as mybir
import concourse.tile as tile
from concourse import bacc
from concourse.bass_utils import run_bass_kernel_spmd

f32 = mybir.dt.float32
bf16 = mybir.dt.bfloat16
AF = mybir.ActivationFunctionType
OP = mybir.AluOpType

L, D, H, V, DFF = 8, 1024, 16, 50257, 4096
HS = D // H          # 64
B, T = 4, 1024
TPC = 512            # tokens per core
P = 128
DC = D // P          # 8 d-chunks
DC2 = 4              # key chunks (128) per sequence half
FC = DFF // P        # 32 dff-chunks
NVC = (V + 511) // 512   # 99 vocab chunks
EPS = 1e-5
VW = H * (HS + 1)    # 1040 (v with ones column per head)

K_SZ = DC * P * TPC            # K staging elems (d-major, own 512 tokens)
V_SZ = 4 * P * VW              # V_aug staging elems
KV_SZ = K_SZ + V_SZ

LAST_EXEC_NS = None
_CACHE = {}


def _install_ntff_hook():
    try:
        import antenv
        try:
            from antenv import axon_hooks  # noqa: F401
            return
        except ImportError:
            pass
        hooks_mod = types.ModuleType("antenv.axon_hooks")
        _hook = [None]
        hooks_mod.set_axon_ntff_profile_hook = lambda h: _hook.__setitem__(0, h)
        hooks_mod.get_axon_ntff_profile_hook = lambda: _hook[0]
        sys.modules["antenv.axon_hooks"] = hooks_mod
        antenv.axon_hooks = hooks_mod
        from trn_agent_boot.trn_boot import _ntff_profile_via_ctypes
        hooks_mod.set_axon_ntff_profile_hook(
            _ntff_profile_via_ctypes("/opt/axon/libaxon_pjrt.so"))
    except Exception:
        pass


def _build():
    nc = bacc.Bacc(None, target_bir_lowering=False, debug=False)

    xembT = nc.dram_tensor("xembT", [D, TPC], f32, kind="ExternalInput")
    wq = nc.dram_tensor("wq", [L, P, DC, D], bf16, kind="ExternalInput")
    wk = nc.dram_tensor("wk", [L, P, DC, D], bf16, kind="ExternalInput")
    wv = nc.dram_tensor("wv", [L, P, DC, D], bf16, kind="ExternalInput")
    wo = nc.dram_tensor("wo", [L, P, DC, D], bf16, kind="ExternalInput")
    w1 = nc.dram_tensor("w1", [L, FC, P, DC, P], bf16, kind="ExternalInput")
    w2 = nc.dram_tensor("w2", [L, 4, DC, P, 8, P], bf16, kind="ExternalInput")
    wlm = nc.dram_tensor("wlm", [NVC, P, DC, 512], bf16, kind="ExternalInput")
    bo_d = nc.dram_tensor("bo", [L, P, DC], f32, kind="ExternalInput")
    b1_d = nc.dram_tensor("b1", [L, P, FC], f32, kind="ExternalInput")
    b2_d = nc.dram_tensor("b2", [L, P, DC], f32, kind="ExternalInput")
    trimask_d = nc.dram_tensor("trimask", [P, P], bf16, kind="ExternalInput")
    ebias_d = nc.dram_tensor("ebias", [P, 1], f32, kind="ExternalInput")
    out_d = nc.dram_tensor("out", [TPC, NVC * 512], bf16, kind="ExternalOutput")

    kv_loc = nc.dram_tensor("kv_loc", [KV_SZ], bf16)
    kv_gat = nc.dram_tensor("kv_gat", [2, KV_SZ], bf16)
    groups = [[0, 1], [2, 3], [4, 5], [6, 7]]

    with tile.TileContext(nc) as tc:
        with (
            tc.tile_pool(name="pool", bufs=1) as pool,
            tc.tile_pool(name="wpool", bufs=2) as wpool,
            tc.tile_pool(name="lnbf", bufs=3) as lnbf,
            tc.tile_pool(name="act", bufs=2) as actp,
            tc.tile_pool(name="sexp_s", bufs=4) as sexp_s,
            tc.tile_pool(name="small", bufs=4) as small,
            tc.tile_pool(name="outp", bufs=3) as outp,
            tc.tile_pool(name="pmm", bufs=4, space="PSUM") as pmm,
            tc.tile_pool(name="pexp", bufs=2, space="PSUM") as pexp,
        ):
            # ---- persistent tiles
            x = pool.tile([P, DC, TPC], f32, name="x")
            kst = pool.tile([P, DC, TPC], bf16, name="kst")
            vst = pool.tile([P, 4, VW], bf16, name="vst")
            krem = pool.tile([P, DC, TPC], bf16, name="krem")
            vrem = pool.tile([P, 4, VW], bf16, name="vrem")
            qbf = pool.tile([P, DC, TPC], bf16, name="qbf")
            obf = pool.tile([P, DC, TPC], bf16, name="obf")
            avA = pool.tile([HS + 1, H, TPC], bf16, name="avA")
            r = pool.tile([P, 8, TPC], bf16, name="r")
            trimask = pool.tile([P, P], bf16, name="trimask")
            ebias_t = pool.tile([P, 1], f32, name="ebias_t")
            ones128b = pool.tile([P, 1], bf16, name="ones128b")
            ones1b = pool.tile([1, P], bf16, name="ones1b")
            nc.vector.memset(ones128b[:], 1.0)
            nc.vector.memset(ones1b[:], 1.0)
            oneshs = pool.tile([P, HS], bf16, name="oneshs")
            nc.vector.memset(oneshs[:], 1.0)
            eps_t = pool.tile([1, 1], f32, name="eps_t")
            nc.vector.memset(eps_t[:], EPS)
            # ones columns of the V staging buffer are constant; set once
            nc.vector.memset(vst[:], 1.0)
            nc.sync.dma_start(trimask[:], trimask_d[:])
            nc.sync.dma_start(ebias_t[:], ebias_d[:])
            xv = xembT.rearrange("(c p) t -> p c t", p=P)
            for c in range(DC):
                nc.sync.dma_start(x[:, c, :], xv[:, c, :])

            def stat_tile(nm):
                # sx in bank 0, sq in bank 1 of one 2-bank psum tile
                return pexp.tile([P, 2, TPC], f32, tag="px", name=f"stat_{nm}")

            def stat_chunk(xin, c, st, first, nm):
                """cast chunk c to bf16, square, accumulate column sums."""
                xbf = lnbf.tile([P, TPC], bf16, tag="xbf", name=f"xbf_{nm}_{c}")
                sqbf = lnbf.tile([P, TPC], bf16, tag="sqbf", name=f"sqbf_{nm}_{c}")
                nc.vector.tensor_copy(xbf[:], xin[:, c, :])
                nc.gpsimd.tensor_mul(sqbf[:], xbf[:], xbf[:])
                nc.tensor.matmul(st[0:1, 0, :], ones128b[:], xbf[:],
                                 start=first, stop=(c == DC - 1))
                nc.tensor.matmul(st[0:1, 1, :], ones128b[:], sqbf[:],
                                 start=first, stop=(c == DC - 1))

            def ln_apply(xin, st, out_bf, nm):
                """rstd = 1/sqrt((sq - sx*mu)/D + eps); out = x*rstd - mu*rstd."""
                sx = st[0:1, 0, :]
                sq = st[0:1, 1, :]
                mu = small.tile([1, TPC], f32, tag="smu", bufs=1, name=f"mu_{nm}")
                nc.vector.tensor_scalar_mul(mu[:], sx, 1.0 / D)
                t1 = small.tile([1, TPC], f32, tag="sm", bufs=2, name=f"t1_{nm}")
                nc.vector.tensor_mul(t1[:], sx, mu[:])
                v = small.tile([1, TPC], f32, tag="sm", bufs=2, name=f"v_{nm}")
                nc.vector.tensor_sub(v[:], sq, t1[:])
                std = small.tile([1, TPC], f32, tag="sm2", bufs=1, name=f"std_{nm}")
                nc.scalar.activation(std[:], v[:], AF.Sqrt, bias=eps_t[:], scale=1.0 / D)
                # keepwarm: a throwaway fp32 matmul on the fresh std keeps the
                # PE's activity window busy through this serial chain so HAM
                # doesn't re-throttle the clock.
                kw = pmm.tile([P, TPC], f32, tag="mm", name=f"kw_{nm}")
                nc.tensor.matmul(kw[:, 0:P], std[0:1, 0:P], std[0:1, 0:P],
                                 start=True, stop=True)
                rm = small.tile([1, 2, TPC], f32, tag="rm", bufs=1, name=f"rm_{nm}")
                nc.vector.reciprocal_approx_fast(out=rm[0:1, 0, :], in_=std[:])
                nc.vector.tensor_mul(rm[0:1, 1, :], mu[:], rm[0:1, 0, :])
                rmbf = small.tile([1, 2, TPC], bf16, tag="rmbf", bufs=1, name=f"rmbf_{nm}")
                nc.vector.tensor_copy(rmbf[:], rm[:])
                bcast = pexp.tile([P, 2, TPC], f32, tag="px", name=f"bcast_{nm}")
                nc.tensor.matmul(bcast[:, 0, :], ones1b[:], rmbf[0:1, 0, :],
                                 start=True, stop=True)
                nc.tensor.matmul(bcast[:, 1, :], ones1b[:], rmbf[0:1, 1, :],
                                 start=True, stop=True)
                for c in range(DC):
                    nc.vector.tensor_mul(out_bf[:, c, :], xin[:, c, :], bcast[:, 0, :])
                    nc.vector.tensor_sub(out_bf[:, c, :], out_bf[:, c, :], bcast[:, 1, :])

            # layer-0 LN1 stats from the embedding
            st = stat_tile("l0")
            for c in range(DC):
                stat_chunk(x, c, st, first=(c == 0), nm="l0")

            for li in range(L):
                # ---------- LN1 -> hbf ----------
                hbf = actp.tile([P, DC, TPC], bf16, tag="a", name=f"hbf_{li}")
                ln_apply(x, st, hbf, f"l1_{li}")

                # ---------- K projection (feeds the exchange) ----------
                # output chunks in pairs alternating psum banks: hides both
                # the LDWEIGHTS and the same-bank accumulate-drain stall.
                wk_t = wpool.tile([P, DC, D], bf16, tag="w", name=f"wk_{li}")
                nc.sync.dma_start(wk_t[:], wk[li])
                for m2 in range(4):
                    pk = [pmm.tile([P, TPC], f32, tag="mm", name=f"kps_{li}_{m2}_{i}")
                          for i in range(2)]
                    for c in range(DC):
                        for i in range(2):
                            m = 2 * m2 + i
                            nc.tensor.matmul(pk[i][:], wk_t[:, c, m * P:(m + 1) * P],
                                             hbf[:, c, :],
                                             start=(c == 0), stop=(c == DC - 1))
                    for i in range(2):
                        nc.scalar.activation(kst[:, 2 * m2 + i, :], pk[i][:], AF.Copy)
                # stage K early (overlaps the V projection)
                nc.sync.dma_start(
                    kv_loc[0:K_SZ].rearrange("(p c t) -> p c t", c=DC, t=TPC), kst[:])

                # ---------- V projection (x chunks stationary; mh inner) ----------
                wv_t = wpool.tile([P, DC, D], bf16, tag="w", name=f"wv_{li}")
                nc.sync.dma_start(wv_t[:], wv[li])
                for tc4 in range(4):
                    pv = [pmm.tile([P, TPC], f32, tag="mm", name=f"vps_{li}_{tc4}_{mh}")
                          for mh in range(2)]
                    for c in range(DC):
                        for mh in range(2):
                            nc.tensor.matmul(
                                pv[mh][:], hbf[:, c, tc4 * P:(tc4 + 1) * P],
                                wv_t[:, c, mh * 512:(mh + 1) * 512],
                                start=(c == 0), stop=(c == DC - 1))
                    dst = vst[:, tc4, :].rearrange("p (h e) -> p h e", e=HS + 1)
                    for mh in range(2):
                        nc.vector.tensor_copy(
                            dst[:, mh * 8:(mh + 1) * 8, 0:HS],
                            pv[mh][:].rearrange("p (h e) -> p h e", e=HS))

                nc.sync.dma_start(
                    kv_loc[K_SZ:KV_SZ].rearrange("(p c t) -> p c t", c=4, t=VW), vst[:])
                nc.gpsimd.collective_compute(
                    "AllGather", OP.bypass, replica_groups=groups,
                    ins=[kv_loc[:]], outs=[kv_gat[:]])

                # ---------- Q projection (overlaps the collective) ----------
                wq_t = wpool.tile([P, DC, D], bf16, tag="w", name=f"wq_{li}")
                nc.sync.dma_start(wq_t[:], wq[li])
                for m2 in range(4):
                    pq = [pmm.tile([P, TPC], f32, tag="mm", name=f"qps_{li}_{m2}_{i}")
                          for i in range(2)]
                    for c in range(DC):
                        for i in range(2):
                            m = 2 * m2 + i
                            nc.tensor.matmul(pq[i][:], wq_t[:, c, m * P:(m + 1) * P],
                                             hbf[:, c, :],
                                             start=(c == 0), stop=(c == DC - 1))
                    for i in range(2):
                        nc.scalar.activation(qbf[:, 2 * m2 + i, :], pq[i][:], AF.Copy)

                # prefetch wo + reduced-KV readback (slot 0 = half-0 member)
                wo_t = wpool.tile([P, DC, D], bf16, tag="w", name=f"wo_{li}")
                nc.sync.dma_start(wo_t[:], wo[li])
                # chunked readback: the first phase-B head pair only needs
                # krem chunk 0 / vrem head-cols 0:130, so don't gate it on the
                # full 2.1MB transfer.
                kv_gk = kv_gat[0, 0:K_SZ].rearrange("(p c t) -> p c t", c=DC, t=TPC)
                for cchunk in range(4):
                    nc.sync.dma_start(krem[:, 2 * cchunk:2 * cchunk + 2, :],
                                      kv_gk[:, 2 * cchunk:2 * cchunk + 2, :])
                kv_gv = kv_gat[0, K_SZ:KV_SZ].rearrange("(p c t) -> p c t", c=4, t=VW)
                for vh in range(2):
                    nc.sync.dma_start(vrem[:, :, vh * 520:(vh + 1) * 520],
                                      kv_gv[:, :, vh * 520:(vh + 1) * 520])

                # ---------- attention ----------
                # denominators parked at 32-aligned partitions: tile g holds
                # heads 4g..4g+3 at partition bases {0,32,64,96}
                den4 = [small.tile([P, TPC], f32, tag="den4", name=f"den4_{li}_{g}")
                        for g in range(4)]
                rden4 = [small.tile([P, TPC], f32, tag="rden4", name=f"rden4_{li}_{g}")
                         for g in range(4)]
                rdbf4 = [small.tile([P, TPC], bf16, tag="rdbf", name=f"rdbf_{li}_{g}")
                         for g in range(4)]

                # phase A: own half, causally trimmed, heads processed in
                # row-complementary pairs (even head rows 0-63, odd 64-127)
                # so their S matmuls run concurrently in the PE array.
                # per head one 2-bank psum tile, reused kt01 -> exp -> kt23:
                #   pass 1: kt0 -> [0:512], kt1 -> [512:896]
                #   pass 2: kt2 -> [0:256], kt3 -> [256:384]
                for h2 in range(H // 2):
                    hs2 = [2 * h2, 2 * h2 + 1]
                    pt = [pexp.tile([P, 2, TPC], f32, tag="px", name=f"pA_{li}_{h}")
                          for h in hs2]
                    pfl = [t.rearrange("p a t -> p (a t)") for t in pt]
                    sxAs = [sexp_s.tile([P, 1024], bf16, tag="sx", name=f"sxA_{li}_{h}")
                            for h in hs2]
                    sxBs = [sexp_s.tile([P, 1024], bf16, tag="sx", name=f"sxB_{li}_{h}")
                            for h in hs2]
                    for kt in range(2):
                        for j, h in enumerate(hs2):
                            hp = (h % 2) * HS
                            hc = h // 2
                            lo = kt * 512
                            nc.tensor.matmul(
                                pfl[j][:, lo:lo + 512 - kt * 128],
                                kst[hp:hp + HS, hc, kt * P:(kt + 1) * P],
                                qbf[hp:hp + HS, hc, kt * P:TPC],
                                start=True, stop=True)
                    for j, h in enumerate(hs2):
                        nc.scalar.activation(sxAs[j][:, 0:896], pfl[j][:, 0:896],
                                             AF.Exp, scale=HS ** -0.5)
                    for kt in range(2, 4):
                        for j, h in enumerate(hs2):
                            hp = (h % 2) * HS
                            hc = h // 2
                            lo = (kt - 2) * 256
                            nc.tensor.matmul(
                                pfl[j][:, lo:lo + 512 - kt * 128],
                                kst[hp:hp + HS, hc, kt * P:(kt + 1) * P],
                                qbf[hp:hp + HS, hc, kt * P:TPC],
                                start=True, stop=True)
                    for j, h in enumerate(hs2):
                        nc.scalar.activation(sxBs[j][:, 0:384], pfl[j][:, 0:384],
                                             AF.Exp, scale=HS ** -0.5)
                    for j, h in enumerate(hs2):
                        sxA, sxB = sxAs[j], sxBs[j]
                        nc.gpsimd.tensor_mul(sxA[:, 0:128], sxA[:, 0:128], trimask[:])
                        nc.gpsimd.tensor_mul(sxA[:, 512:640], sxA[:, 512:640], trimask[:])
                        nc.gpsimd.tensor_mul(sxB[:, 0:128], sxB[:, 0:128], trimask[:])
                        nc.gpsimd.tensor_mul(sxB[:, 256:384], sxB[:, 256:384], trimask[:])
                    # AV for the pair, interleaved across two psum slots
                    pav = [pmm.tile([P, TPC], f32, tag="mm", name=f"avA_{li}_{h}")
                           for h in hs2]
                    av_rng = [(0, 512, 0), (512, 896, 128), (1024 + 0, 1024 + 256, 256),
                              (1024 + 256, 1024 + 384, 384)]
                    for kt in range(4):
                        lo, hi, olo = av_rng[kt]
                        for j, h in enumerate(hs2):
                            src = sxAs[j] if lo < 1024 else sxBs[j]
                            slo, shi = lo % 1024, (hi - 1) % 1024 + 1
                            nc.tensor.matmul(pav[j][0:HS + 1, olo:512],
                                             vst[:, kt, h * 65:h * 65 + 65],
                                             src[:, slo:shi],
                                             start=(kt == 0), stop=(kt == 3))
                    for j, h in enumerate(hs2):
                        nc.vector.tensor_copy(avA[:, h, :], pav[j][0:HS + 1, :])

                # keepwarm burst bridging the collective wait: the PE would
                # otherwise idle ~25us here, HAM halves its clock, and all of
                # phase B then runs at 1.2GHz. ~115 junk matmuls (~25us warm)
                # keep the activity window busy until krem lands.
                dmy = pexp.tile([P, 2, TPC], f32, tag="px", name=f"dmy_{li}")
                for t in range(185 if li == 0 else 152):
                    nc.tensor.matmul(dmy[:, t % 2, :], kst[:, t % DC, 0:P],
                                     kst[:, (t + 1) % DC, :], start=True, stop=True)

                # phase B: remote half (zeroed on half-0 via the exp bias),
                # same head-pair structure; kt01 -> exp -> kt23 tile reuse.
                for h2 in range(H // 2):
                    hs2 = [2 * h2, 2 * h2 + 1]
                    pt = [pexp.tile([P, 2, TPC], f32, tag="px", name=f"pB_{li}_{h}")
                          for h in hs2]
                    pfl = [t.rearrange("p a t -> p (a t)") for t in pt]
                    sxAs = [sexp_s.tile([P, 1024], bf16, tag="sx", name=f"sxBA_{li}_{h}")
                            for h in hs2]
                    sxBs = [sexp_s.tile([P, 1024], bf16, tag="sx", name=f"sxBB_{li}_{h}")
                            for h in hs2]
                    for kt in range(2):
                        for j, h in enumerate(hs2):
                            hp = (h % 2) * HS
                            hc = h // 2
                            nc.tensor.matmul(pfl[j][:, kt * 512:(kt + 1) * 512],
                                             krem[hp:hp + HS, hc, kt * P:(kt + 1) * P],
                                             qbf[hp:hp + HS, hc, :], start=True, stop=True)
                    for j, h in enumerate(hs2):
                        nc.scalar.activation(sxAs[j][:], pfl[j][:], AF.Exp,
                                             bias=ebias_t[:], scale=HS ** -0.5)
                    for kt in range(2):
                        for j, h in enumerate(hs2):
                            hp = (h % 2) * HS
                            hc = h // 2
                            nc.tensor.matmul(pfl[j][:, kt * 512:(kt + 1) * 512],
                                             krem[hp:hp + HS, hc, (2 + kt) * P:(3 + kt) * P],
                                             qbf[hp:hp + HS, hc, :], start=True, stop=True)
                    for j, h in enumerate(hs2):
                        nc.scalar.activation(sxBs[j][:], pfl[j][:], AF.Exp,
                                             bias=ebias_t[:], scale=HS ** -0.5)
                    pav = [pmm.tile([P, TPC], f32, tag="mm", name=f"avB_{li}_{h}")
                           for h in hs2]
                    for kt in range(4):
                        for j, h in enumerate(hs2):
                            src = sxAs[j] if kt < 2 else sxBs[j]
                            nc.tensor.matmul(
                                pav[j][0:HS + 1, :], vrem[:, kt, h * 65:h * 65 + 65],
                                src[:, (kt % 2) * 512:(kt % 2 + 1) * 512],
                                start=(kt == 0), stop=(kt == 3))
                    for j, h in enumerate(hs2):
                        nc.vector.tensor_add(avA[0:HS, h, :], pav[j][0:HS, :],
                                             avA[0:HS, h, :])
                        g, j4 = h // 4, (h % 4) * 32
                        nc.vector.tensor_add(den4[g][j4:j4 + 1, :], pav[j][HS:HS + 1, :],
                                             avA[HS:HS + 1, h, :])

                # small warm bridge over the denominator chain
                dm2 = pexp.tile([P, 2, TPC], f32, tag="px", name=f"dmy2_{li}")
                for t in range(20):
                    nc.tensor.matmul(dm2[:, t % 2, :], kst[:, t % DC, 0:P],
                                     kst[:, (t + 1) % DC, :], start=True, stop=True)

                # batched softmax denominators: 4 partition-parallel approx
                # reciprocals cover all 16 heads (4 each at bases 0/32/64/96)
                for g in range(4):
                    nc.vector.reciprocal_approx_fast(out=rden4[g][:], in_=den4[g][:])
                    nc.vector.tensor_copy(rdbf4[g][:], rden4[g][:])
                for h in range(H):
                    hp = (h % 2) * HS
                    hc = h // 2
                    g, j = h // 4, (h % 4) * 32
                    bc = pmm.tile([P, TPC], f32, tag="mm", name=f"bc_{li}_{h}")
                    nc.tensor.matmul(bc[0:HS, :], oneshs[j:j + 1, 0:HS],
                                     rdbf4[g][j:j + 1, :],
                                     start=True, stop=True,
                                     tile_position=(j, 0))
                    bcs = outp.tile([HS, TPC], bf16, tag="bcs", bufs=2, name=f"bcs_{li}_{h}")
                    nc.vector.tensor_copy(bcs[:], bc[0:HS, :])
                    nc.gpsimd.tensor_mul(obf[hp:hp + HS, hc, :], avA[0:HS, h, :], bcs[:])

                # ---------- output projection + residual + LN2 stats ----------
                bo_t = small.tile([P, DC, 1], f32, tag="bias", name=f"bo_{li}")
                nc.sync.dma_start(bo_t[:], bo_d[li][:, :, None])
                st = stat_tile(f"a_{li}")
                for m2 in range(4):
                    po = [pmm.tile([P, TPC], f32, tag="mm", name=f"ops2_{li}_{m2}_{i}")
                          for i in range(2)]
                    for c in range(DC):
                        for i in range(2):
                            m = 2 * m2 + i
                            nc.tensor.matmul(po[i][:], wo_t[:, c, m * P:(m + 1) * P],
                                             obf[:, c, :],
                                             start=(c == 0), stop=(c == DC - 1))
                    for i in range(2):
                        m = 2 * m2 + i
                        nc.vector.scalar_tensor_tensor(
                            x[:, m, :], po[i][:], bo_t[:, m], x[:, m, :],
                            op0=OP.add, op1=OP.add)
                        stat_chunk(x, m, st, first=(m == 0), nm=f"a{li}")

                # ---------- LN2 + MLP ----------
                h2 = actp.tile([P, DC, TPC], bf16, tag="a", name=f"h2_{li}")
                ln_apply(x, st, h2, f"l2_{li}")

                b1_t = small.tile([P, FC, 1], f32, tag="b1", name=f"b1_{li}")
                nc.sync.dma_start(b1_t[:], b1_d[li][:, :, None])
                b2_t = small.tile([P, DC, 1], f32, tag="bias", name=f"b2_{li}")
                nc.sync.dma_start(b2_t[:], b2_d[li][:, :, None])
                st = stat_tile(f"m_{li}")
                for qr in range(4):
                    for mf2 in range(4):
                        mf = qr * 8 + 2 * mf2
                        w1_t = wpool.tile([P, 2, DC, P], bf16, tag="wmlp",
                                          name=f"w1_{li}_{mf}")
                        nc.sync.dma_start(
                            w1_t[:], w1[li, mf:mf + 2].rearrange("f p c n -> p f c n"))
                        p1 = [pmm.tile([P, TPC], f32, tag="mm", name=f"mps_{li}_{mf}_{i}")
                              for i in range(2)]
                        for c in range(DC):
                            for i in range(2):
                                nc.tensor.matmul(p1[i][:], w1_t[:, i, c, :], h2[:, c, :],
                                                 start=(c == 0), stop=(c == DC - 1))
                        for i in range(2):
                            nc.scalar.activation(r[:, 2 * mf2 + i, :], p1[i][:], AF.Relu,
                                                 bias=b1_t[:, mf + i], scale=1.0)
                    for m2 in range(4):
                        m = 2 * m2
                        w2_t = wpool.tile([P, 2, 8, P], bf16, tag="wmlp",
                                          name=f"w2_{li}_{qr}_{m}")
                        nc.sync.dma_start(
                            w2_t[:], w2[li, qr, m:m + 2].rearrange("f p c n -> p f c n"))
                        p2 = [pmm.tile([P, TPC], f32, tag="mm", name=f"m2ps_{li}_{qr}_{m}_{i}")
                              for i in range(2)]
                        for c in range(8):
                            for i in range(2):
                                nc.tensor.matmul(p2[i][:], w2_t[:, i, c, :], r[:, c, :],
                                                 start=(c == 0), stop=(c == 7))
                        for i in range(2):
                            if qr == 0:
                                nc.vector.scalar_tensor_tensor(
                                    x[:, m + i, :], p2[i][:], b2_t[:, m + i], x[:, m + i, :],
                                    op0=OP.add, op1=OP.add)
                            elif qr < 3:
                                nc.vector.tensor_add(x[:, m + i, :], x[:, m + i, :], p2[i][:])
                            else:
                                nc.vector.tensor_add(x[:, m + i, :], x[:, m + i, :], p2[i][:])
                                stat_chunk(x, m + i, st, first=(m + i == 0), nm=f"m{li}")

            # ---------- final LN + LM head ----------
            xf = actp.tile([P, DC, TPC], bf16, tag="a", name="xf")
            ln_apply(x, st, xf, "lf")

            # vocab tiles in pairs: back-to-back matmuls with identical lhsT
            # (xf chunk) so codegen can keep the PE weights loaded.
            for vp in range(50):
                nvc_here = 2 if vp < 49 else 1
                wl_t = wpool.tile([P, 2, DC, 512], bf16, tag="w", name=f"wlm_{vp}")
                nc.sync.dma_start(
                    wl_t[:, 0:nvc_here],
                    wlm[2 * vp:2 * vp + nvc_here].rearrange("v p c n -> p v c n"))
                for tc4 in range(4):
                    pv = [pmm.tile([P, 512], f32, tag="mm", name=f"lmps_{vp}_{tc4}_{i}")
                          for i in range(nvc_here)]
                    for c in range(DC):
                        for i in range(nvc_here):
                            nc.tensor.matmul(pv[i][:], xf[:, c, tc4 * P:(tc4 + 1) * P],
                                             wl_t[:, i, c, :],
                                             start=(c == 0), stop=(c == DC - 1))
                    ot = outp.tile([P, 2, 512], bf16, tag="o", bufs=2, name=f"ot_{vp}_{tc4}")
                    for i in range(nvc_here):
                        if i == 0:
                            nc.vector.tensor_copy(ot[:, i, :], pv[i][:])
                        else:
                            nc.scalar.activation(ot[:, i, :], pv[i][:], AF.Copy)
                    nc.sync.dma_start(
                        out_d[tc4 * P:(tc4 + 1) * P,
                              vp * 1024:vp * 1024 + nvc_here * 512],
                        ot[:, 0:nvc_here].rearrange("p v n -> p (v n)"))

    nc.compile()
    return nc


def kernel(**inputs):
    global LAST_EXEC_NS
    _install_ntff_hook()

    gi = {k: np.asarray(v) for k, v in inputs.items()}
    # this kernel folds LN scales into weights and assumes zero biases where
    # skipping them is an approximation; verify those assumptions hold
    assert not np.any(gi["blm"]), "nonzero blm not supported by this kernel"
    for k in ("ln1_b", "ln2_b", "lnf_b"):
        assert not np.any(gi[k]), f"nonzero {k} not supported"

    if "nc" not in _CACHE:
        _CACHE["nc"] = _build()
    nc = _CACHE["nc"]

    idx = gi["idx"].astype(np.int64)
    xemb = gi["wte"][idx] + gi["wpe"][:T][None, :, :]      # [B, T, D] fp32

    def pack_sq(w, lnw=None):   # [L, 1024, N] -> [L, 128, 8, N]
        w = np.asarray(w, np.float32)
        if lnw is not None:
            w = w * np.asarray(lnw, np.float32)[:, :, None]
        Lw, Kw, Nw = w.shape
        return np.ascontiguousarray(
            w.reshape(Lw, DC, P, Nw).transpose(0, 2, 1, 3).astype(ml_dtypes.bfloat16))

    w1s = np.asarray(gi["w1"], np.float32) * np.asarray(gi["ln2_w"], np.float32)[:, :, None]
    w1p = w1s.reshape(L, DC, P, FC, P).transpose(0, 3, 2, 1, 4)   # [L,FC,P,DC,P]
    w1p = np.ascontiguousarray(w1p.astype(ml_dtypes.bfloat16))
    w2p = gi["w2"].reshape(L, 4, 8, P, DC, P).transpose(0, 1, 4, 3, 2, 5)  # [L,4,DC,P,8,P]
    w2p = np.ascontiguousarray(w2p.astype(ml_dtypes.bfloat16))
    wlmp = np.zeros((D, NVC * 512), np.float32)
    wlmp[:, :V] = np.asarray(gi["wlm"], np.float32) * np.asarray(gi["lnf_w"], np.float32)[:, None]
    wlmp = wlmp.reshape(DC, P, NVC, 512).transpose(2, 1, 0, 3)         # [NVC,P,DC,512]
    wlmp = np.ascontiguousarray(wlmp.astype(ml_dtypes.bfloat16))

    def packv(v):  # [.., N] -> [.., P, N//P]
        v = np.asarray(v, np.float32)
        nch = v.shape[-1] // P
        return np.ascontiguousarray(
            v.reshape(v.shape[:-1] + (nch, P)).swapaxes(-1, -2))

    # causal mask of one diagonal [128,128] key/query block
    tri = (np.arange(P)[:, None] <= np.arange(P)[None, :]).astype(np.float32)
    tri = tri.astype(ml_dtypes.bfloat16)

    shared = dict(
        wq=pack_sq(gi["wq"], gi["ln1_w"]), wk=pack_sq(gi["wk"], gi["ln1_w"]),
        wv=pack_sq(gi["wv"], gi["ln1_w"]), wo=pack_sq(gi["wo"]),
        w1=w1p, w2=w2p, wlm=wlmp,
        bo=packv(gi["bo"]), b1=packv(gi["b1"]), b2=packv(gi["b2"]),
        trimask=tri,
    )

    in_maps = []
    for c in range(8):
        b, half = c // 2, c % 2
        sl = slice(half * TPC, (half + 1) * TPC)
        im = dict(shared)
        im["xembT"] = np.ascontiguousarray(xemb[b, sl].T, dtype=np.float32)
        im["ebias"] = np.full((P, 1), -30000.0 * (1 - half), np.float32)
        in_maps.append(im)

    res = run_bass_kernel_spmd(nc, in_maps, list(range(8)),
                               trace=bool(os.environ.get("BASS_TRACE")))
    LAST_EXEC_NS = res.exec_time_ns
    _CACHE["res"] = res

    out = np.empty((B, T, V), np.float32)
    for c in range(8):
        b, half = c // 2, c % 2
        out[b, half * TPC:(half + 1) * TPC] = \
            res.results[c]["out"][:, :V].astype(np.float32)
    return out


# revision 27
# speedup vs baseline: 1.1713x; 1.1713x over previous
"""GPT-2 (L=8, D=1024, H=16, V=50257, B=4, T=1024) forward on 8 TRN2 NeuronCores.

Sharding: core c handles batch b=c//2, sequence half h=c%2 (512 tokens).
Weights replicated (bf16). Per layer, the pair exchanges K/V via a gated
AllGather; slot 0 of the gather is the half-0 member's K/V, which is the
remote half every core's phase B uses (self-masked via the exp bias on
half-0 cores: exp(s - 30000) == 0).

Perf rework vs the original baseline (5.17ms -> ~3.6ms):
- attention phase A causally trimmed: S/exp/AV only computed for q >= kt*128;
  mask multiply only on the [128,128] diagonal block of each key chunk.
- exp batched: 2 activation instructions per head-phase reading multi-bank
  PSUM spans instead of 4; attention heads processed in row-complementary
  pairs so their K=64 S matmuls run concurrently in the PE array.
- ALL dense GEMM chains (K/Q/O proj, MLP, LM head) interleave two output
  chunks across two PSUM banks: back-to-back matmuls accumulating into the
  same bank serialize with their LDWEIGHTS (~320ns/MM for N=512); alternating
  banks hides both and reaches ~233ns/MM.
- keepwarm matmul bursts bridge the K/V-exchange wait (~25us) and the LN /
  denominator serial chains: any PE idle gap > ~3.4us makes the HAM clock
  gate halve the PE clock, which previously kept entire attention phases at
  1.2GHz (measured 1.45ms at K=4/8 -> 0.75ms with the bursts).
- LN chain: fused DVE ops + reciprocal_approx_fast instead of Sqrt+DVE
  reciprocal (iterative divide, 3-4us on the critical path); LN stats
  accumulate into spare PSUM banks via ones-matmuls during the producing
  GEMM's drain loop.
- softmax denominators: reciprocal_approx_fast, bf16 broadcast matmuls,
  explicit tile_position=(96,0) for the 4th head of each den group.
- LM head: vocab tiles in pairs sharing the stationary xf chunk, bf16 logits
  (cast to f32 on host) halving the output write traffic.
- vst ones-columns memset once; PSUM->SBUF drains spread across engines.
Run-to-run note: the chip toggles between 2.4GHz and a ~2.0GHz P0 power
state; identical kernels measure 3.64-3.87ms depending on that state.
"""

import os
import sys
import types

import numpy as np
import ml_dtypes

import concourse.bass as bass
import concourse.mybir as mybir
import concourse.tile as tile
from concourse import bacc
from concourse.bass_utils import run_bass_kernel_spmd

f32 = mybir.dt.float32
bf16 = mybir.dt.bfloat16
AF = mybir.ActivationFunctionType
OP = mybir.AluOpType

L, D, H, V, DFF = 8, 1024, 16, 50257, 4096
HS = D // H          # 64
B, T = 4, 1024
TPC = 512            # tokens per core
P = 128
DC = D // P          # 8 d-chunks
DC2 = 4              # key chunks (128) per sequence half
FC = DFF // P        # 32 dff-chunks
NVC = (V + 511) // 512   # 99 vocab chunks
EPS = 1e-5
VW = H * (HS + 1)    # 1040 (v with ones column per head)

K_SZ = DC * P * TPC            # K staging elems (d-major, own 512 tokens)
V_SZ = 4 * P * VW              # V_aug staging elems
KV_SZ = K_SZ + V_SZ

LAST_EXEC_NS = None
_CACHE = {}


def _install_ntff_hook():
    try:
        import antenv
        try:
            from antenv import axon_hooks  # noqa: F401
            return
        except ImportError:
            pass
        hooks_mod = types.ModuleType("antenv.axon_hooks")
        _hook = [None]
        hooks_mod.set_axon_ntff_profile_hook = lambda h: _hook.__setitem__(0, h)
        hooks_mod.get_axon_ntff_profile_hook = lambda: _hook[0]
        sys.modules["antenv.axon_hooks"] = hooks_mod
        antenv.axon_hooks = hooks_mod
        from trn_agent_boot.trn_boot import _ntff_profile_via_ctypes
        hooks_mod.set_axon_ntff_profile_hook(
            _ntff_profile_via_ctypes("/opt/axon/libaxon_pjrt.so"))
    except Exception:
        pass


def _build():
    nc = bacc.Bacc(None, target_bir_lowering=False, debug=False)

    xembT = nc.dram_tensor("xembT", [D, TPC], f32, kind="ExternalInput")
    wq = nc.dram_tensor("wq", [L, P, DC, D], bf16, kind="ExternalInput")
    wk = nc.dram_tensor("wk", [L, P, DC, D], bf16, kind="ExternalInput")
    wv = nc.dram_tensor("wv", [L, P, DC, D], bf16, kind="ExternalInput")
    wo = nc.dram_tensor("wo", [L, P, DC, D], bf16, kind="ExternalInput")
    w1 = nc.dram_tensor("w1", [L, FC, P, DC, P], bf16, kind="ExternalInput")
    w2 = nc.dram_tensor("w2", [L, 4, DC, P, 8, P], bf16, kind="ExternalInput")
    wlm = nc.dram_tensor("wlm", [NVC, P, DC, 512], bf16, kind="ExternalInput")
    bo_d = nc.dram_tensor("bo", [L, P, DC], f32, kind="ExternalInput")
    b1_d = nc.dram_tensor("b1", [L, P, FC], f32, kind="ExternalInput")
    b2_d = nc.dram_tensor("b2", [L, P, DC], f32, kind="ExternalInput")
    trimask_d = nc.dram_tensor("trimask", [P, P], bf16, kind="ExternalInput")
    ebias_d = nc.dram_tensor("ebias", [P, 1], f32, kind="ExternalInput")
    out_d = nc.dram_tensor("out", [TPC, NVC * 512], bf16, kind="ExternalOutput")

    kv_loc = nc.dram_tensor("kv_loc", [KV_SZ], bf16)
    kv_gat = nc.dram_tensor("kv_gat", [2, KV_SZ], bf16)
    groups = [[0, 1], [2, 3], [4, 5], [6, 7]]

    with tile.TileContext(nc) as tc:
        with (
            tc.tile_pool(name="pool", bufs=1) as pool,
            tc.tile_pool(name="wpool", bufs=2) as wpool,
            tc.tile_pool(name="lnbf", bufs=3) as lnbf,
            tc.tile_pool(name="act", bufs=2) as actp,
            tc.tile_pool(name="sexp_s", bufs=4) as sexp_s,
            tc.tile_pool(name="small", bufs=4) as small,
            tc.tile_pool(name="outp", bufs=3) as outp,
            tc.tile_pool(name="pmm", bufs=4, space="PSUM") as pmm,
            tc.tile_pool(name="pexp", bufs=2, space="PSUM") as pexp,
        ):
            # ---- persistent tiles
            x = pool.tile([P, DC, TPC], f32, name="x")
            kst = pool.tile([P, DC, TPC], bf16, name="kst")
            vst = pool.tile([P, 4, VW], bf16, name="vst")
            krem = pool.tile([P, DC, TPC], bf16, name="krem")
            vrem = pool.tile([P, 4, VW], bf16, name="vrem")
            qbf = pool.tile([P, DC, TPC], bf16, name="qbf")
            obf = pool.tile([P, DC, TPC], bf16, name="obf")
            avA = pool.tile([HS + 1, H, TPC], bf16, name="avA")
            r = pool.tile([P, 8, TPC], bf16, name="r")
            trimask = pool.tile([P, P], bf16, name="trimask")
            ebias_t = pool.tile([P, 1], f32, name="ebias_t")
            ones128b = pool.tile([P, 1], bf16, name="ones128b")
            ones1b = pool.tile([1, P], bf16, name="ones1b")
            nc.vector.memset(ones128b[:], 1.0)
            nc.vector.memset(ones1b[:], 1.0)
            oneshs = pool.tile([P, HS], bf16, name="oneshs")
            nc.vector.memset(oneshs[:], 1.0)
            eps_t = pool.tile([1, 1], f32, name="eps_t")
            nc.vector.memset(eps_t[:], EPS)
            # ones columns of the V staging buffer are constant; set once
            nc.vector.memset(vst[:], 1.0)
            nc.sync.dma_start(trimask[:], trimask_d[:])
            nc.sync.dma_start(ebias_t[:], ebias_d[:])
            xv = xembT.rearrange("(c p) t -> p c t", p=P)
            for c in range(DC):
                nc.sync.dma_start(x[:, c, :], xv[:, c, :])

            def stat_tile(nm):
                # sx in bank 0, sq in bank 1 of one 2-bank psum tile
                return pexp.tile([P, 2, TPC], f32, tag="px", name=f"stat_{nm}")

            def stat_chunk(xin, c, st, first, nm):
                """cast chunk c to bf16, square, accumulate column sums."""
                xbf = lnbf.tile([P, TPC], bf16, tag="xbf", name=f"xbf_{nm}_{c}")
                sqbf = lnbf.tile([P, TPC], bf16, tag="sqbf", name=f"sqbf_{nm}_{c}")
                nc.vector.tensor_copy(xbf[:], xin[:, c, :])
                nc.gpsimd.tensor_mul(sqbf[:], xbf[:], xbf[:])
                nc.tensor.matmul(st[0:1, 0, :], ones128b[:], xbf[:],
                                 start=first, stop=(c == DC - 1))
                nc.tensor.matmul(st[0:1, 1, :], ones128b[:], sqbf[:],
                                 start=first, stop=(c == DC - 1))

            def ln_apply(xin, st, out_bf, nm):
                """rstd = 1/sqrt((sq - sx*mu)/D + eps); out = x*rstd - mu*rstd."""
                sx = st[0:1, 0, :]
                sq = st[0:1, 1, :]
                mu = small.tile([1, TPC], f32, tag="smu", bufs=1, name=f"mu_{nm}")
                nc.vector.tensor_scalar_mul(mu[:], sx, 1.0 / D)
                t1 = small.tile([1, TPC], f32, tag="sm", bufs=2, name=f"t1_{nm}")
                nc.vector.tensor_mul(t1[:], sx, mu[:])
                v = small.tile([1, TPC], f32, tag="sm", bufs=2, name=f"v_{nm}")
                nc.vector.tensor_sub(v[:], sq, t1[:])
                std = small.tile([1, TPC], f32, tag="sm2", bufs=1, name=f"std_{nm}")
                nc.scalar.activation(std[:], v[:], AF.Sqrt, bias=eps_t[:], scale=1.0 / D)
                # keepwarm: a throwaway fp32 matmul on the fresh std keeps the
                # PE's activity window busy through this serial chain so HAM
                # doesn't re-throttle the clock.
                kw = pmm.tile([P, TPC], f32, tag="mm", name=f"kw_{nm}")
                nc.tensor.matmul(kw[:, 0:P], std[0:1, 0:P], std[0:1, 0:P],
                                 start=True, stop=True)
                rm = small.tile([1, 2, TPC], f32, tag="rm", bufs=1, name=f"rm_{nm}")
                nc.vector.reciprocal_approx_fast(out=rm[0:1, 0, :], in_=std[:])
                nc.vector.tensor_mul(rm[0:1, 1, :], mu[:], rm[0:1, 0, :])
                rmbf = small.tile([1, 2, TPC], bf16, tag="rmbf", bufs=1, name=f"rmbf_{nm}")
                nc.vector.tensor_copy(rmbf[:], rm[:])
                bcast = pexp.tile([P, 2, TPC], f32, tag="px", name=f"bcast_{nm}")
                nc.tensor.matmul(bcast[:, 0, :], ones1b[:], rmbf[0:1, 0, :],
                                 start=True, stop=True)
                nc.tensor.matmul(bcast[:, 1, :], ones1b[:], rmbf[0:1, 1, :],
                                 start=True, stop=True)
                for c in range(DC):
                    nc.vector.tensor_mul(out_bf[:, c, :], xin[:, c, :], bcast[:, 0, :])
                    nc.vector.tensor_sub(out_bf[:, c, :], out_bf[:, c, :], bcast[:, 1, :])

            # layer-0 LN1 stats from the embedding
            st = stat_tile("l0")
            for c in range(DC):
                stat_chunk(x, c, st, first=(c == 0), nm="l0")

            for li in range(L):
                # ---------- LN1 -> hbf ----------
                hbf = actp.tile([P, DC, TPC], bf16, tag="a", name=f"hbf_{li}")
                ln_apply(x, st, hbf, f"l1_{li}")

                # ---------- K projection (feeds the exchange) ----------
                # output chunks in pairs alternating psum banks: hides both
                # the LDWEIGHTS and the same-bank accumulate-drain stall.
                wk_t = wpool.tile([P, DC, D], bf16, tag="w", name=f"wk_{li}")
                nc.sync.dma_start(wk_t[:], wk[li])
                for m2 in range(4):
                    pk = [pmm.tile([P, TPC], f32, tag="mm", name=f"kps_{li}_{m2}_{i}")
                          for i in range(2)]
                    for c in range(DC):
                        for i in range(2):
                            m = 2 * m2 + i
                            nc.tensor.matmul(pk[i][:], wk_t[:, c, m * P:(m + 1) * P],
                                             hbf[:, c, :],
                                             start=(c == 0), stop=(c == DC - 1))
                    for i in range(2):
                        nc.scalar.activation(kst[:, 2 * m2 + i, :], pk[i][:], AF.Copy)
                # stage K early (overlaps the V projection)
                nc.sync.dma_start(
                    kv_loc[0:K_SZ].rearrange("(p c t) -> p c t", c=DC, t=TPC), kst[:])

                # ---------- V projection (x chunks stationary; mh inner) ----------
                wv_t = wpool.tile([P, DC, D], bf16, tag="w", name=f"wv_{li}")
                nc.sync.dma_start(wv_t[:], wv[li])
                for tc4 in range(4):
                    pv = [pmm.tile([P, TPC], f32, tag="mm", name=f"vps_{li}_{tc4}_{mh}")
                          for mh in range(2)]
                    for c in range(DC):
                        for mh in range(2):
                            nc.tensor.matmul(
                                pv[mh][:], hbf[:, c, tc4 * P:(tc4 + 1) * P],
                                wv_t[:, c, mh * 512:(mh + 1) * 512],
                                start=(c == 0), stop=(c == DC - 1))
                    dst = vst[:, tc4, :].rearrange("p (h e) -> p h e", e=HS + 1)
                    for mh in range(2):
                        nc.vector.tensor_copy(
                            dst[:, mh * 8:(mh + 1) * 8, 0:HS],
                            pv[mh][:].rearrange("p (h e) -> p h e", e=HS))

                nc.sync.dma_start(
                    kv_loc[K_SZ:KV_SZ].rearrange("(p c t) -> p c t", c=4, t=VW), vst[:])
                nc.gpsimd.collective_compute(
                    "AllGather", OP.bypass, replica_groups=groups,
                    ins=[kv_loc[:]], outs=[kv_gat[:]])

                # ---------- Q projection (overlaps the collective) ----------
                wq_t = wpool.tile([P, DC, D], bf16, tag="w", name=f"wq_{li}")
                nc.sync.dma_start(wq_t[:], wq[li])
                for m2 in range(4):
                    pq = [pmm.tile([P, TPC], f32, tag="mm", name=f"qps_{li}_{m2}_{i}")
                          for i in range(2)]
                    for c in range(DC):
                        for i in range(2):
                            m = 2 * m2 + i
                            nc.tensor.matmul(pq[i][:], wq_t[:, c, m * P:(m + 1) * P],
                                             hbf[:, c, :],
                                             start=(c == 0), stop=(c == DC - 1))
                    for i in range(2):
                        nc.scalar.activation(qbf[:, 2 * m2 + i, :], pq[i][:], AF.Copy)

                # prefetch wo + reduced-KV readback (slot 0 = half-0 member)
                wo_t = wpool.tile([P, DC, D], bf16, tag="w", name=f"wo_{li}")
                nc.sync.dma_start(wo_t[:], wo[li])
                # chunked readback: the first phase-B head pair only needs
                # krem chunk 0 / vrem head-cols 0:130, so don't gate it on the
                # full 2.1MB transfer.
                kv_gk = kv_gat[0, 0:K_SZ].rearrange("(p c t) -> p c t", c=DC, t=TPC)
                for cchunk in range(4):
                    nc.sync.dma_start(krem[:, 2 * cchunk:2 * cchunk + 2, :],
                                      kv_gk[:, 2 * cchunk:2 * cchunk + 2, :])
                kv_gv = kv_gat[0, K_SZ:KV_SZ].rearrange("(p c t) -> p c t", c=4, t=VW)
                for vh in range(2):
                    nc.sync.dma_start(vrem[:, :, vh * 520:(vh + 1) * 520],
                                      kv_gv[:, :, vh * 520:(vh + 1) * 520])

                # ---------- attention ----------
                # denominators parked at 32-aligned partitions: tile g holds
                # heads 4g..4g+3 at partition bases {0,32,64,96}
                den4 = [small.tile([P, TPC], f32, tag="den4", name=f"den4_{li}_{g}")
                        for g in range(4)]
                rden4 = [small.tile([P, TPC], f32, tag="rden4", name=f"rden4_{li}_{g}")
                         for g in range(4)]
                rdbf4 = [small.tile([P, TPC], bf16, tag="rdbf", name=f"rdbf_{li}_{g}")
                         for g in range(4)]

                # phase A: own half, causally trimmed, heads processed in
                # row-complementary pairs (even head rows 0-63, odd 64-127)
                # so their S matmuls run concurrently in the PE array.
                # per head one 2-bank psum tile, reused kt01 -> exp -> kt23:
                #   pass 1: kt0 -> [0:512], kt1 -> [512:896]
                #   pass 2: kt2 -> [0:256], kt3 -> [256:384]
                for h2 in range(H // 2):
                    hs2 = [2 * h2, 2 * h2 + 1]
                    pt = [pexp.tile([P, 2, TPC], f32, tag="px", name=f"pA_{li}_{h}")
                          for h in hs2]
                    pfl = [t.rearrange("p a t -> p (a t)") for t in pt]
                    sxAs = [sexp_s.tile([P, 1024], bf16, tag="sx", name=f"sxA_{li}_{h}")
                            for h in hs2]
                    sxBs = [sexp_s.tile([P, 1024], bf16, tag="sx", name=f"sxB_{li}_{h}")
                            for h in hs2]
                    for kt in range(2):
                        for j, h in enumerate(hs2):
                            hp = (h % 2) * HS
                            hc = h // 2
                            lo = kt * 512
                            nc.tensor.matmul(
                                pfl[j][:, lo:lo + 512 - kt * 128],
                                kst[hp:hp + HS, hc, kt * P:(kt + 1) * P],
                                qbf[hp:hp + HS, hc, kt * P:TPC],
                                start=True, stop=True)
                    for j, h in enumerate(hs2):
                        nc.scalar.activation(sxAs[j][:, 0:896], pfl[j][:, 0:896],
                                             AF.Exp, scale=HS ** -0.5)
                    kwa = pmm.tile([P, TPC], f32, tag="mm", name=f"kwa_{li}_{h2}")
                    for t in range(2):
                        nc.tensor.matmul(kwa[:], kst[:, (h2 + t) % DC, 0:P],
                                         kst[:, (h2 + t + 1) % DC, :],
                                         start=True, stop=True)
                    for kt in range(2, 4):
                        for j, h in enumerate(hs2):
                            hp = (h % 2) * HS
                            hc = h // 2
                            lo = (kt - 2) * 256
                            nc.tensor.matmul(
                                pfl[j][:, lo:lo + 512 - kt * 128],
                                kst[hp:hp + HS, hc, kt * P:(kt + 1) * P],
                                qbf[hp:hp + HS, hc, kt * P:TPC],
                                start=True, stop=True)
                    for j, h in enumerate(hs2):
                        nc.scalar.activation(sxBs[j][:, 0:384], pfl[j][:, 0:384],
                                             AF.Exp, scale=HS ** -0.5)
                    for j, h in enumerate(hs2):
                        sxA, sxB = sxAs[j], sxBs[j]
                        nc.gpsimd.tensor_mul(sxA[:, 0:128], sxA[:, 0:128], trimask[:])
                        nc.gpsimd.tensor_mul(sxA[:, 512:640], sxA[:, 512:640], trimask[:])
                        nc.gpsimd.tensor_mul(sxB[:, 0:128], sxB[:, 0:128], trimask[:])
                        nc.gpsimd.tensor_mul(sxB[:, 256:384], sxB[:, 256:384], trimask[:])
                    # AV for the pair, interleaved across two psum slots
                    pav = [pmm.tile([P, TPC], f32, tag="mm", name=f"avA_{li}_{h}")
                           for h in hs2]
                    av_rng = [(0, 512, 0), (512, 896, 128), (1024 + 0, 1024 + 256, 256),
                              (1024 + 256, 1024 + 384, 384)]
                    for kt in range(4):
                        lo, hi, olo = av_rng[kt]
                        for j, h in enumerate(hs2):
                            src = sxAs[j] if lo < 1024 else sxBs[j]
                            slo, shi = lo % 1024, (hi - 1) % 1024 + 1
                            nc.tensor.matmul(pav[j][0:HS + 1, olo:512],
                                             vst[:, kt, h * 65:h * 65 + 65],
                                             src[:, slo:shi],
                                             start=(kt == 0), stop=(kt == 3))
                    for j, h in enumerate(hs2):
                        nc.vector.tensor_copy(avA[:, h, :], pav[j][0:HS + 1, :])

                # keepwarm burst bridging the collective wait: the PE would
                # otherwise idle ~25us here, HAM halves its clock, and all of
                # phase B then runs at 1.2GHz. ~115 junk matmuls (~25us warm)
                # keep the activity window busy until krem lands.
                dmy = pexp.tile([P, 2, TPC], f32, tag="px", name=f"dmy_{li}")
                for t in range(185 if li == 0 else 152):
                    nc.tensor.matmul(dmy[:, t % 2, :], kst[:, t % DC, 0:P],
                                     kst[:, (t + 1) % DC, :], start=True, stop=True)

                # phase B: remote half (zeroed on half-0 via the exp bias),
                # same head-pair structure; kt01 -> exp -> kt23 tile reuse.
                for h2 in range(H // 2):
                    hs2 = [2 * h2, 2 * h2 + 1]
                    pt = [pexp.tile([P, 2, TPC], f32, tag="px", name=f"pB_{li}_{h}")
                          for h in hs2]
                    pfl = [t.rearrange("p a t -> p (a t)") for t in pt]
                    sxAs = [sexp_s.tile([P, 1024], bf16, tag="sx", name=f"sxBA_{li}_{h}")
                            for h in hs2]
                    sxBs = [sexp_s.tile([P, 1024], bf16, tag="sx", name=f"sxBB_{li}_{h}")
                            for h in hs2]
                    for kt in range(2):
                        for j, h in enumerate(hs2):
                            hp = (h % 2) * HS
                            hc = h // 2
                            nc.tensor.matmul(pfl[j][:, kt * 512:(kt + 1) * 512],
                                             krem[hp:hp + HS, hc, kt * P:(kt + 1) * P],
                                             qbf[hp:hp + HS, hc, :], start=True, stop=True)
                    for j, h in enumerate(hs2):
                        nc.scalar.activation(sxAs[j][:], pfl[j][:], AF.Exp,
                                             bias=ebias_t[:], scale=HS ** -0.5)
                    kwb = pmm.tile([P, TPC], f32, tag="mm", name=f"kwb_{li}_{h2}")
                    for t in range(2):
                        nc.tensor.matmul(kwb[:], kst[:, (h2 + t) % DC, 0:P],
                                         kst[:, (h2 + t + 1) % DC, :],
                                         start=True, stop=True)
                    for kt in range(2):
                        for j, h in enumerate(hs2):
                            hp = (h % 2) * HS
                            hc = h // 2
                            nc.tensor.matmul(pfl[j][:, kt * 512:(kt + 1) * 512],
                                             krem[hp:hp + HS, hc, (2 + kt) * P:(3 + kt) * P],
                                             qbf[hp:hp + HS, hc, :], start=True, stop=True)
                    for j, h in enumerate(hs2):
                        nc.scalar.activation(sxBs[j][:], pfl[j][:], AF.Exp,
                                             bias=ebias_t[:], scale=HS ** -0.5)
                    pav = [pmm.tile([P, TPC], f32, tag="mm", name=f"avB_{li}_{h}")
                           for h in hs2]
                    for kt in range(4):
                        for j, h in enumerate(hs2):
                            src = sxAs[j] if kt < 2 else sxBs[j]
                            nc.tensor.matmul(
                                pav[j][0:HS + 1, :], vrem[:, kt, h * 65:h * 65 + 65],
                                src[:, (kt % 2) * 512:(kt % 2 + 1) * 512],
                                start=(kt == 0), stop=(kt == 3))
                    for j, h in enumerate(hs2):
                        nc.vector.tensor_add(avA[0:HS, h, :], pav[j][0:HS, :],
                                             avA[0:HS, h, :])
                        g, j4 = h // 4, (h % 4) * 32
                        nc.vector.tensor_add(den4[g][j4:j4 + 1, :], pav[j][HS:HS + 1, :],
                                             avA[HS:HS + 1, h, :])

                # small warm bridge over the denominator chain
                dm2 = pexp.tile([P, 2, TPC], f32, tag="px", name=f"dmy2_{li}")
                for t in range(8):
                    nc.tensor.matmul(dm2[:, t % 2, :], kst[:, t % DC, 0:P],
                                     kst[:, (t + 1) % DC, :], start=True, stop=True)

                # batched softmax denominators: 4 partition-parallel approx
                # reciprocals cover all 16 heads (4 each at bases 0/32/64/96)
                for g in range(4):
                    nc.vector.reciprocal_approx_fast(out=rden4[g][:], in_=den4[g][:])
                    nc.vector.tensor_copy(rdbf4[g][:], rden4[g][:])
                for h in range(H):
                    hp = (h % 2) * HS
                    hc = h // 2
                    g, j = h // 4, (h % 4) * 32
                    bc = pmm.tile([P, TPC], f32, tag="mm", name=f"bc_{li}_{h}")
                    nc.tensor.matmul(bc[0:HS, :], oneshs[j:j + 1, 0:HS],
                                     rdbf4[g][j:j + 1, :],
                                     start=True, stop=True,
                                     tile_position=(j, 0))
                    bcs = outp.tile([HS, TPC], bf16, tag="bcs", bufs=2, name=f"bcs_{li}_{h}")
                    nc.vector.tensor_copy(bcs[:], bc[0:HS, :])
                    nc.gpsimd.tensor_mul(obf[hp:hp + HS, hc, :], avA[0:HS, h, :], bcs[:])

                # ---------- output projection + residual + LN2 stats ----------
                bo_t = small.tile([P, DC, 1], f32, tag="bias", name=f"bo_{li}")
                nc.sync.dma_start(bo_t[:], bo_d[li][:, :, None])
                st = stat_tile(f"a_{li}")
                for m2 in range(4):
                    po = [pmm.tile([P, TPC], f32, tag="mm", name=f"ops2_{li}_{m2}_{i}")
                          for i in range(2)]
                    for c in range(DC):
                        for i in range(2):
                            m = 2 * m2 + i
                            nc.tensor.matmul(po[i][:], wo_t[:, c, m * P:(m + 1) * P],
                                             obf[:, c, :],
                                             start=(c == 0), stop=(c == DC - 1))
                    for i in range(2):
                        m = 2 * m2 + i
                        nc.vector.scalar_tensor_tensor(
                            x[:, m, :], po[i][:], bo_t[:, m], x[:, m, :],
                            op0=OP.add, op1=OP.add)
                        stat_chunk(x, m, st, first=(m == 0), nm=f"a{li}")

                # ---------- LN2 + MLP ----------
                h2 = actp.tile([P, DC, TPC], bf16, tag="a", name=f"h2_{li}")
                ln_apply(x, st, h2, f"l2_{li}")

                b1_t = small.tile([P, FC, 1], f32, tag="b1", name=f"b1_{li}")
                nc.sync.dma_start(b1_t[:], b1_d[li][:, :, None])
                b2_t = small.tile([P, DC, 1], f32, tag="bias", name=f"b2_{li}")
                nc.sync.dma_start(b2_t[:], b2_d[li][:, :, None])
                st = stat_tile(f"m_{li}")
                for qr in range(4):
                    for mf2 in range(4):
                        mf = qr * 8 + 2 * mf2
                        w1_t = wpool.tile([P, 2, DC, P], bf16, tag="wmlp",
                                          name=f"w1_{li}_{mf}")
                        nc.sync.dma_start(
                            w1_t[:], w1[li, mf:mf + 2].rearrange("f p c n -> p f c n"))
                        p1 = [pmm.tile([P, TPC], f32, tag="mm", name=f"mps_{li}_{mf}_{i}")
                              for i in range(2)]
                        for c in range(DC):
                            for i in range(2):
                                nc.tensor.matmul(p1[i][:], w1_t[:, i, c, :], h2[:, c, :],
                                                 start=(c == 0), stop=(c == DC - 1))
                        for i in range(2):
                            nc.scalar.activation(r[:, 2 * mf2 + i, :], p1[i][:], AF.Relu,
                                                 bias=b1_t[:, mf + i], scale=1.0)
                    for m2 in range(4):
                        m = 2 * m2
                        w2_t = wpool.tile([P, 2, 8, P], bf16, tag="wmlp",
                                          name=f"w2_{li}_{qr}_{m}")
                        nc.sync.dma_start(
                            w2_t[:], w2[li, qr, m:m + 2].rearrange("f p c n -> p f c n"))
                        p2 = [pmm.tile([P, TPC], f32, tag="mm", name=f"m2ps_{li}_{qr}_{m}_{i}")
                              for i in range(2)]
                        for c in range(8):
                            for i in range(2):
                                nc.tensor.matmul(p2[i][:], w2_t[:, i, c, :], r[:, c, :],
                                                 start=(c == 0), stop=(c == 7))
                        for i in range(2):
                            if qr == 0:
                                nc.vector.scalar_tensor_tensor(
                                    x[:, m + i, :], p2[i][:], b2_t[:, m + i], x[:, m + i, :],
                                    op0=OP.add, op1=OP.add)
                            elif qr < 3:
                                nc.vector.tensor_add(x[:, m + i, :], x[:, m + i, :], p2[i][:])
                            else:
                                nc.vector.tensor_add(x[:, m + i, :], x[:, m + i, :], p2[i][:])
                                stat_chunk(x, m + i, st, first=(m + i == 0), nm=f"m{li}")

            # ---------- final LN + LM head ----------
            xf = actp.tile([P, DC, TPC], bf16, tag="a", name="xf")
            ln_apply(x, st, xf, "lf")

            # vocab tiles in pairs: back-to-back matmuls with identical lhsT
            # (xf chunk) so codegen can keep the PE weights loaded.
            for vp in range(50):
                nvc_here = 2 if vp < 49 else 1
                wl_t = wpool.tile([P, 2, DC, 512], bf16, tag="w", name=f"wlm_{vp}")
                nc.sync.dma_start(
                    wl_t[:, 0:nvc_here],
                    wlm[2 * vp:2 * vp + nvc_here].rearrange("v p c n -> p v c n"))
                for tc4 in range(4):
                    pv = [pmm.tile([P, 512], f32, tag="mm", name=f"lmps_{vp}_{tc4}_{i}")
                          for i in range(nvc_here)]
                    for c in range(DC):
                        for i in range(nvc_here):
                            nc.tensor.matmul(pv[i][:], xf[:, c, tc4 * P:(tc4 + 1) * P],
                                             wl_t[:, i, c, :],
                                             start=(c == 0), stop=(c == DC - 1))
                    ot = outp.tile([P, 2, 512], bf16, tag="o", bufs=2, name=f"ot_{vp}_{tc4}")
                    for i in range(nvc_here):
                        if i == 0:
                            nc.vector.tensor_copy(ot[:, i, :], pv[i][:])
                        else:
                            nc.scalar.activation(ot[:, i, :], pv[i][:], AF.Copy)
                    nc.sync.dma_start(
                        out_d[tc4 * P:(tc4 + 1) * P,
                              vp * 1024:vp * 1024 + nvc_here * 512],
                        ot[:, 0:nvc_here].rearrange("p v n -> p (v n)"))

    nc.compile()
    return nc


def kernel(**inputs):
    global LAST_EXEC_NS
    _install_ntff_hook()

    gi = {k: np.asarray(v) for k, v in inputs.items()}
    # this kernel folds LN scales into weights and assumes zero biases where
    # skipping them is an approximation; verify those assumptions hold
    assert not np.any(gi["blm"]), "nonzero blm not supported by this kernel"
    for k in ("ln1_b", "ln2_b", "lnf_b"):
        assert not np.any(gi[k]), f"nonzero {k} not supported"

    if "nc" not in _CACHE:
        _CACHE["nc"] = _build()
    nc = _CACHE["nc"]

    idx = gi["idx"].astype(np.int64)
    xemb = gi["wte"][idx] + gi["wpe"][:T][None, :, :]      # [B, T, D] fp32

    def pack_sq(w, lnw=None):   # [L, 1024, N] -> [L, 128, 8, N]
        w = np.asarray(w, np.float32)
        if lnw is not None:
            w = w * np.asarray(lnw, np.float32)[:, :, None]
        Lw, Kw, Nw = w.shape
        return np.ascontiguousarray(
            w.reshape(Lw, DC, P, Nw).transpose(0, 2, 1, 3).astype(ml_dtypes.bfloat16))

    w1s = np.asarray(gi["w1"], np.float32) * np.asarray(gi["ln2_w"], np.float32)[:, :, None]
    w1p = w1s.reshape(L, DC, P, FC, P).transpose(0, 3, 2, 1, 4)   # [L,FC,P,DC,P]
    w1p = np.ascontiguousarray(w1p.astype(ml_dtypes.bfloat16))
    w2p = gi["w2"].reshape(L, 4, 8, P, DC, P).transpose(0, 1, 4, 3, 2, 5)  # [L,4,DC,P,8,P]
    w2p = np.ascontiguousarray(w2p.astype(ml_dtypes.bfloat16))
    wlmp = np.zeros((D, NVC * 512), np.float32)
    wlmp[:, :V] = np.asarray(gi["wlm"], np.float32) * np.asarray(gi["lnf_w"], np.float32)[:, None]
    wlmp = wlmp.reshape(DC, P, NVC, 512).transpose(2, 1, 0, 3)         # [NVC,P,DC,512]
    wlmp = np.ascontiguousarray(wlmp.astype(ml_dtypes.bfloat16))

    def packv(v):  # [.., N] -> [.., P, N//P]
        v = np.asarray(v, np.float32)
        nch = v.shape[-1] // P
        return np.ascontiguousarray(
            v.reshape(v.shape[:-1] + (nch, P)).swapaxes(-1, -2))

    # causal mask of one diagonal [128,128] key/query block
    tri = (np.arange(P)[:, None] <= np.arange(P)[None, :]).astype(np.float32)
    tri = tri.astype(ml_dtypes.bfloat16)

    shared = dict(
        wq=pack_sq(gi["wq"], gi["ln1_w"]), wk=pack_sq(gi["wk"], gi["ln1_w"]),
        wv=pack_sq(gi["wv"], gi["ln1_w"]), wo=pack_sq(gi["wo"]),
        w1=w1p, w2=w2p, wlm=wlmp,
        bo=packv(gi["bo"]), b1=packv(gi["b1"]), b2=packv(gi["b2"]),
        trimask=tri,
    )

    in_maps = []
    for c in range(8):
        b, half = c // 2, c % 2
        sl = slice(half * TPC, (half + 1) * TPC)
        im = dict(shared)
        im["xembT"] = np.ascontiguousarray(xemb[b, sl].T, dtype=np.float32)
        im["ebias"] = np.full((P, 1), -30000.0 * (1 - half), np.float32)
        in_maps.append(im)

    res = run_bass_kernel_spmd(nc, in_maps, list(range(8)),
                               trace=bool(os.environ.get("BASS_TRACE")))
    LAST_EXEC_NS = res.exec_time_ns
    _CACHE["res"] = res

    out = np.empty((B, T, V), np.float32)
    for c in range(8):
        b, half = c // 2, c % 2
        out[b, half * TPC:(half + 1) * TPC] = \
            res.results[c]["out"][:, :V].astype(np.float32)
    return out
